# revision 20
# baseline (speedup 1.0000x reference)
"""Trainium2 Bass kernel for nn_AttentionLayer (cross-attention + FF + LayerNorm).

V2 strategy (data-parallel over batch, 2 per core):
  - bf16 on-chip activations/weights for projections + attention (full PE
    rate at any moving dim, 2x DVE, half DMA); f32r for the reversion
    (wr, xf) and the residual/LN path so the dominant error terms stay f32.
  - Inputs loaded FEATURE-major directly via DMA-transpose (XBAR), removing
    all phase-1 PE transposes and PSUM->SBUF copies.
  - wr loaded once into a const pool (not per batch), prefetched during
    attention of batch 0.
  - Phase 2 software-pipelined: the softmax tail of head i-1 (denominator,
    reciprocal, broadcast, V^T@E) is emitted inside head i's projection
    matmuls, so PE never stalls on Act/DVE round trips (keeps the PE
    p-state at full clock).
  - Softmax un-normalized in [key, query] layout; denominator via ones-row
    matmul; normalization folded into the PSUM->SBUF move of x (TT mult
    with the PE-broadcast reciprocal, both operands in PSUM).
  - LayerNorm: stats computed feature-major with ones-column matmuls
    (sum, sum-of-squares), rstd = exp(-0.5*ln(var+eps)) so every Act
    function lives in one act-table set (no table reloads); the normalize
    is fused into the PSUM->SBUF copies after the transpose back to
    token-major (per-partition scale/bias).
  - V bias folded into the reversion bias host-side: breff = br + bv @ wr.
"""

import os
import sys

import numpy as np

# ---- problem constants (hardcoded per contract) ----
B_TOTAL = 16
N_CORES = 8
B = B_TOTAL // N_CORES  # per-core batch
LT, DT = 512, 768       # text tokens / dim
LI, DI = 576, 1024      # image tokens / dim
H, NH, HD = 2048, 8, 256
FF = 128
ISCALE = 1.0 / 16.0     # 1/sqrt(HD)
NPAIR = B * NH          # 16 (batch, head) pairs per core
ITC = [(t, 128 if t < 4 else LI - 512) for t in range(5)]  # image tok chunks
NCD = DT // 128         # 6
NCI = DI // 128         # 8
NT = LT // 128          # 4

_BUILD_CACHE: dict = {}


def _ensure_import_path():
    try:
        import concourse  # noqa: F401
    except ModuleNotFoundError:
        for p in ("/opt/trn_rl_repo", "/root/.axon_site/_ro/trn_rl_repo"):
            if os.path.isdir(p) and p not in sys.path:
                sys.path.insert(0, p)


def build_module(apply_gamma: bool = False):
    key = ("v2", apply_gamma)
    if key in _BUILD_CACHE:
        return _BUILD_CACHE[key]
    _ensure_import_path()
    from contextlib import ExitStack

    import concourse.bacc as bacc
    import concourse.bass as bass  # noqa: F401
    import concourse.mybir as mybir
    import concourse.tile as tile
    from concourse.masks import make_identity

    f32 = mybir.dt.float32
    f32r = mybir.dt.float32r
    bf16 = mybir.dt.bfloat16
    AF = mybir.ActivationFunctionType
    ALU = mybir.AluOpType

    def r(ap):
        return ap.bitcast(f32r)

    nc = bacc.Bacc("TRN2", target_bir_lowering=False, debug=False, num_devices=N_CORES)

    text = nc.dram_tensor("text", [B, LT, DT], bf16, kind="ExternalInput").ap()
    image = nc.dram_tensor("image", [B, LI, DI], bf16, kind="ExternalInput").ap()
    wq = nc.dram_tensor("wq", [DT, H], bf16, kind="ExternalInput").ap()
    wk = nc.dram_tensor("wk", [DI, H], bf16, kind="ExternalInput").ap()
    wv = nc.dram_tensor("wv", [DI, H], bf16, kind="ExternalInput").ap()
    wr = nc.dram_tensor("wr", [H, DT], f32, kind="ExternalInput").ap()
    w1 = nc.dram_tensor("w1", [DT, FF], bf16, kind="ExternalInput").ap()
    w2 = nc.dram_tensor("w2", [FF, DT], bf16, kind="ExternalInput").ap()
    bq = nc.dram_tensor("bq", [H], f32, kind="ExternalInput").ap()
    bk = nc.dram_tensor("bk", [H], f32, kind="ExternalInput").ap()
    b1 = nc.dram_tensor("b1", [FF], f32, kind="ExternalInput").ap()
    b2 = nc.dram_tensor("b2", [DT], f32, kind="ExternalInput").ap()
    breff = nc.dram_tensor("breff", [DT], f32, kind="ExternalInput").ap()
    gamma = nc.dram_tensor("gamma", [DT], f32, kind="ExternalInput").ap()
    beta = nc.dram_tensor("beta", [DT], f32, kind="ExternalInput").ap()
    out = nc.dram_tensor("out", [B, LT, DT], f32, kind="ExternalOutput").ap()
    scr = nc.dram_tensor("scr", [B, 2, LT], f32, kind="Internal").ap()

    def bcast_row(src, parts, n):
        return bass.AP(tensor=src.tensor, offset=src.offset, ap=[[0, parts], *src.ap])

    with tile.TileContext(nc) as tc, ExitStack() as ctx:
        const = ctx.enter_context(tc.tile_pool(name="const", bufs=1))
        ident = const.tile([128, 128], f32)
        make_identity(nc, ident)
        ident_r = const.tile([128, 128], f32)
        nc.vector.tensor_copy(out=r(ident_r), in_=ident)
        ones_col_b = const.tile([128, 1], bf16)
        nc.vector.memset(ones_col_b, 1.0)
        ones_row_b = const.tile([1, 128], bf16)
        nc.vector.memset(ones_row_b, 1.0)
        ones_tmp = const.tile([128, 1], f32)
        nc.vector.memset(ones_tmp, 1.0)
        ones_col_f = const.tile([128, 1], f32)
        nc.vector.tensor_copy(out=r(ones_col_f), in_=ones_tmp)
        eps_t = const.tile([128, 1], f32)
        nc.vector.memset(eps_t, 1e-5)

        bq_sb = const.tile([128, H // 128], f32)
        nc.sync.dma_start(out=bq_sb, in_=bq.rearrange("(j p) -> p j", p=128))
        bk_sb = const.tile([128, H // 128], f32)
        nc.sync.dma_start(out=bk_sb, in_=bk.rearrange("(j p) -> p j", p=128))
        b1_sb = const.tile([128, 1], f32)
        nc.sync.dma_start(out=b1_sb, in_=b1.rearrange("(j p) -> p j", p=128))
        b2_sb = const.tile([128, NCD], f32)
        nc.sync.dma_start(out=b2_sb, in_=b2.rearrange("(j p) -> p j", p=128))
        breff_sb = const.tile([128, NCD], f32)
        nc.sync.dma_start(out=breff_sb, in_=breff.rearrange("(j p) -> p j", p=128))
        w1_sb = const.tile([128, NCD, FF], bf16)
        nc.sync.dma_start(out=w1_sb, in_=w1.rearrange("(c p) n -> p c n", p=128))
        w2_sb = const.tile([128, DT], bf16)
        nc.sync.dma_start(out=w2_sb, in_=w2)
        wr_sb = const.tile([128, H // 128, DT], f32)  # loaded in 4 chunks mid-flight
        if apply_gamma:
            gam_sb = const.tile([128, DT], f32)
            nc.sync.dma_start(out=gam_sb, in_=bcast_row(gamma, 128, DT))
            bet_sb = const.tile([128, DT], f32)
            nc.sync.dma_start(out=bet_sb, in_=bcast_row(beta, 128, DT))

        psum = ctx.enter_context(tc.tile_pool(name="psum", bufs=4, space="PSUM"))
        psd = ctx.enter_context(tc.tile_pool(name="psd", bufs=2, space="PSUM"))
        pstr = ctx.enter_context(tc.tile_pool(name="pstr", bufs=2, space="PSUM"))
        tfp = ctx.enter_context(tc.tile_pool(name="tfp", bufs=2))
        hwp = ctx.enter_context(tc.tile_pool(name="hwp", bufs=2))
        atp = ctx.enter_context(tc.tile_pool(name="atp", bufs=2))
        xfp = ctx.enter_context(tc.tile_pool(name="xfp", bufs=1))
        ofp = ctx.enter_context(tc.tile_pool(name="ofp", bufs=1))
        p5p = ctx.enter_context(tc.tile_pool(name="p5p", bufs=2))

        TFI: dict = {}
        W: dict = {}
        S: dict = {}
        XF: dict = {}
        OF: dict = {}
        SQ: dict = {}

        def load_inputs(b):
            tfs = [tfp.tile([128, LT], bf16, tag=f"textf{c}", name=f"textf{c}")
                   for c in range(NCD)]
            ifs = [tfp.tile([128, LI], bf16, tag=f"imgf{c}", name=f"imgf{c}")
                   for c in range(NCI)]
            for c in range(NCD):
                nc.sync.dma_start_transpose(
                    out=tfs[c], in_=text[b, :, c * 128:(c + 1) * 128])
            for c in range(NCI):
                nc.sync.dma_start_transpose(
                    out=ifs[c], in_=image[b, :, c * 128:(c + 1) * 128])
            TFI[b] = (tfs, ifs)

        def load_weights(i):
            h = i % NH
            wq_h = hwp.tile([128, NCD, HD], bf16, tag="wqh", name="wqh")
            nc.sync.dma_start(
                out=wq_h,
                in_=wq.rearrange("(c p) n -> p c n", p=128)[:, :, h * HD:(h + 1) * HD])
            wk_h = hwp.tile([128, NCI, HD], bf16, tag="wkh", name="wkh")
            nc.sync.dma_start(
                out=wk_h,
                in_=wk.rearrange("(c p) n -> p c n", p=128)[:, :, h * HD:(h + 1) * HD])
            wv_h = hwp.tile([128, NCI, HD], bf16, tag="wvh", name="wvh")
            nc.sync.dma_start(
                out=wv_h,
                in_=wv.rearrange("(c p) n -> p c n", p=128)[:, :, h * HD:(h + 1) * HD])
            W[i] = (wq_h, wk_h, wv_h)

        def load_wr_chunk(cc):
            nc.sync.dma_start(
                out=r(wr_sb[:, cc * 4:(cc + 1) * 4, :]),
                in_=r(wr.rearrange("(c p) n -> p c n", p=128)[:, cc * 4:(cc + 1) * 4, :]))

        def alloc_xf(b):
            XF[b] = [xfp.tile([128, LT], f32, tag=f"xf{c}", name=f"xf{c}")
                     for c in range(H // 128)]

        def emit_Q(i):
            b, h = divmod(i, NH)
            tfs, _ = TFI[b]
            wq_h, _, _ = W[i]
            q_h = atp.tile([128, 2, LT], bf16, tag="qh", name="qh")
            for m in range(2):
                pq = psum.tile([128, LT], f32, tag="ps512", name="ps512")
                for c in range(NCD):
                    nc.tensor.matmul(pq, wq_h[:, c, m * 128:(m + 1) * 128], tfs[c],
                                     start=(c == 0), stop=(c == NCD - 1))
                nc.scalar.activation(
                    out=q_h[:, m, :], in_=pq, func=AF.Identity,
                    bias=bq_sb[:, h * 2 + m:h * 2 + m + 1], scale=1.0)
            S[i] = {"q": q_h}

        def emit_K(i):
            b, h = divmod(i, NH)
            _, ifs = TFI[b]
            _, wk_h, _ = W[i]
            k_h = atp.tile([128, 2, LI], bf16, tag="kh", name="kh")
            for m in range(2):
                for n in range(2):
                    pk = psum.tile([128, 288], f32, tag="ps512", name="ps512")
                    for c in range(NCI):
                        nc.tensor.matmul(
                            pk, wk_h[:, c, m * 128:(m + 1) * 128],
                            ifs[c][:, n * 288:(n + 1) * 288],
                            start=(c == 0), stop=(c == NCI - 1))
                    nc.scalar.activation(
                        out=k_h[:, m, n * 288:(n + 1) * 288], in_=pk,
                        func=AF.Identity,
                        bias=bk_sb[:, h * 2 + m:h * 2 + m + 1], scale=1.0)
            S[i]["k"] = k_h

        def emit_V(i):
            b, h = divmod(i, NH)
            _, ifs = TFI[b]
            _, _, wv_h = W[i]
            v_h = atp.tile([128, 5, HD], bf16, tag="vh", name="vh")
            for t, pt in ITC:
                pv = psum.tile([128, HD], f32, tag="ps512", name="ps512")
                for c in range(NCI):
                    nc.tensor.matmul(
                        pv[:pt], ifs[c][:, t * 128:t * 128 + pt], wv_h[:, c, :],
                        start=(c == 0), stop=(c == NCI - 1))
                nc.vector.tensor_copy(out=v_h[:pt, t, :], in_=pv[:pt])
            S[i]["v"] = v_h

        def emit_scores(i):
            q_h, k_h = S[i]["q"], S[i]["k"]
            e_f = atp.tile([128, 5, LT], bf16, tag="ef", name="ef")
            for t, pt in ITC:
                ps_s = psum.tile([128, LT], f32, tag="ps512", name="ps512")
                for m in range(2):
                    nc.tensor.matmul(
                        ps_s[:pt], k_h[:, m, t * 128:t * 128 + pt], q_h[:, m, :],
                        start=(m == 0), stop=(m == 1))
                nc.scalar.activation(
                    out=e_f[:pt, t, :], in_=ps_s[:pt], func=AF.Exp, scale=ISCALE)
            S[i]["e"] = e_f

        def emit_pd(i):
            e_f = S[i]["e"]
            pd2 = psd.tile([1, LT], f32, tag="psd", name="psd")
            for t, pt in ITC:
                nc.tensor.matmul(pd2[0:1, :], ones_col_b[:pt], e_f[:pt, t, :],
                                 start=(t == 0), stop=(t == 4))
            recip = atp.tile([1, LT], bf16, tag="recip", name="recip")
            with nc.allow_low_precision(reason="softmax recip feeds bf16 matmul"):
                nc.vector.reciprocal(out=recip, in_=pd2[0:1, :])
            S[i]["recip"] = recip

        def emit_pbc(i):
            pbc = psum.tile([128, LT], f32, tag="ps512", name="ps512")
            nc.tensor.matmul(pbc, ones_row_b, S[i]["recip"])
            bcast = atp.tile([128, LT], bf16, tag="bcast", name="bcast")
            nc.scalar.activation(out=bcast, in_=pbc, func=AF.Copy)
            S[i]["bcast"] = bcast

        def emit_px(i):
            b, h = divmod(i, NH)
            v_h, e_f, bcast = S[i]["v"], S[i]["e"], S[i]["bcast"]
            for m in range(2):
                px = psum.tile([128, LT], f32, tag="ps512", name="ps512")
                for t, pt in ITC:
                    nc.tensor.matmul(
                        px, v_h[:pt, t, m * 128:(m + 1) * 128], e_f[:pt, t, :],
                        start=(t == 0), stop=(t == 4))
                nc.vector.tensor_mul(out=r(XF[b][h * 2 + m]), in0=px, in1=bcast)
            S[i] = None  # release references

        def emit_rev(b):
            ofs = [ofp.tile([128, LT], f32, tag=f"of{m}", name=f"of{m}")
                   for m in range(NCD)]
            ofb = [ofp.tile([128, LT], bf16, tag=f"ofb{m}", name=f"ofb{m}")
                   for m in range(NCD)]
            for m in range(NCD):
                po = psum.tile([128, LT], f32, tag="ps512", name="ps512")
                for c in range(H // 128):
                    nc.tensor.matmul(po, r(wr_sb[:, c, m * 128:(m + 1) * 128]),
                                     r(XF[b][c]),
                                     start=(c == 0), stop=(c == H // 128 - 1))
                nc.scalar.activation(
                    out=r(ofs[m]), in_=po, func=AF.Identity,
                    bias=breff_sb[:, m:m + 1], scale=1.0)
                nc.vector.tensor_scalar(
                    out=ofb[m], in0=po, scalar1=breff_sb[:, m:m + 1], scalar2=None,
                    op0=ALU.add)
            OF[b] = (ofs, ofb)

        def emit_ff(b):
            ofs, ofb = OF[b]
            ph = psum.tile([128, LT], f32, tag="ps512", name="ps512")
            for c in range(NCD):
                nc.tensor.matmul(ph, w1_sb[:, c, :], ofb[c],
                                 start=(c == 0), stop=(c == NCD - 1))
            h_sb = p5p.tile([128, LT], bf16, tag="hsb", name="hsb")
            nc.scalar.activation(out=h_sb, in_=ph, func=AF.Relu, bias=b1_sb,
                                 scale=1.0)
            for m in range(NCD):
                pf = psum.tile([128, LT], f32, tag="ps512", name="ps512")
                nc.tensor.matmul(pf, w2_sb[:, m * 128:(m + 1) * 128], h_sb)
                ff_sb = p5p.tile([128, LT], f32, tag="ffsb", name="ffsb")
                nc.scalar.activation(
                    out=ff_sb, in_=pf, func=AF.Identity,
                    bias=b2_sb[:, m:m + 1], scale=1.0)
                nc.vector.tensor_add(out=r(ofs[m]), in0=ofs[m], in1=ff_sb)
                # square for the variance sum, emitted early so Act keeps pace
                sq = ofp.tile([128, LT], bf16, tag=f"sq{m}", name=f"sq{m}")
                nc.scalar.activation(out=sq, in_=ofs[m], func=AF.Square, scale=1.0)
                SQ.setdefault(b, []).append(sq)

        def emit_stats(b):
            ofs, _ = OF[b]
            pstatA = psd.tile([1, LT], f32, tag="psd", name="psd")
            for m in range(NCD):
                nc.tensor.matmul(pstatA, r(ones_col_f), r(ofs[m]),
                                 start=(m == 0), stop=(m == NCD - 1))
            pstatB = psd.tile([1, LT], f32, tag="psd", name="psd")
            for m in range(NCD):
                nc.tensor.matmul(pstatB, ones_col_b, SQ[b][m],
                                 start=(m == 0), stop=(m == NCD - 1))
            srowA = ofp.tile([1, LT], f32, tag="srowA", name="srowA")
            srowB = ofp.tile([1, LT], f32, tag="srowB", name="srowB")
            nc.vector.tensor_copy(out=srowA, in_=pstatA)
            nc.vector.tensor_copy(out=srowB, in_=pstatB)
            # row -> per-partition columns via DRAM roundtrip (SBUF->SBUF
            # partition-crossing DMA mangles data on HW)
            nc.sync.dma_start(out=scr[b, 0:1, :], in_=srowA)
            nc.sync.dma_start(out=scr[b, 1:2, :], in_=srowB)
            stcolA = p5p.tile([128, NT], f32, tag="stcolA", name="stcolA")
            stcolB = p5p.tile([128, NT], f32, tag="stcolB", name="stcolB")
            nc.sync.dma_start(
                out=stcolA, in_=scr[b, 0:1, :].rearrange("a (t p) -> p (a t)", p=128))
            nc.sync.dma_start(
                out=stcolB, in_=scr[b, 1:2, :].rearrange("a (t p) -> p (a t)", p=128))
            stats = []
            for t in range(NT):
                mu_t = p5p.tile([128, 1], f32, tag=f"mu{t}", name=f"mu{t}")
                nc.vector.tensor_scalar(
                    out=mu_t, in0=stcolA[:, t:t + 1], scalar1=1.0 / DT,
                    scalar2=None, op0=ALU.mult)
                musq = p5p.tile([128, 1], f32, tag=f"musq{t}", name=f"musq{t}")
                nc.vector.tensor_mul(out=musq, in0=mu_t, in1=mu_t)
                var_t = p5p.tile([128, 1], f32, tag=f"var{t}", name=f"var{t}")
                nc.vector.scalar_tensor_tensor(
                    out=var_t, in0=stcolB[:, t:t + 1], scalar=1.0 / DT, in1=musq,
                    op0=ALU.mult, op1=ALU.subtract)
                lnv = p5p.tile([128, 1], f32, tag=f"lnv{t}", name=f"lnv{t}")
                nc.scalar.activation(out=lnv, in_=var_t, func=AF.Ln, bias=eps_t,
                                     scale=1.0)
                rstd = p5p.tile([128, 1], f32, tag=f"rstd{t}", name=f"rstd{t}")
                nc.scalar.activation(out=rstd, in_=lnv, func=AF.Exp, scale=-0.5)
                nmr = p5p.tile([128, 1], f32, tag=f"nmr{t}", name=f"nmr{t}")
                nc.vector.tensor_scalar(
                    out=nmr, in0=mu_t, scalar1=rstd, scalar2=-1.0,
                    op0=ALU.mult, op1=ALU.mult)
                stats.append((rstd, nmr))
            OF[b] = (ofs, stats)

        def emit_ln_chunk(b, t):
            ofs, stats = OF[b]
            rstd, nmr = stats[t]
            y = p5p.tile([128, DT], f32, tag="y", name="y")
            for c in range(NCD):
                ptr_ = pstr.tile([128, 128], f32, tag="ptr", name="ptr")
                nc.tensor.transpose(r(ptr_), r(ofs[c][:, t * 128:(t + 1) * 128]),
                                    r(ident_r))
                dst = y[:, c * 128:(c + 1) * 128]
                if c % 2 == 0:
                    nc.scalar.activation(out=dst, in_=ptr_, func=AF.Identity,
                                         bias=nmr, scale=rstd)
                else:
                    nc.vector.tensor_scalar(
                        out=dst, in0=ptr_, scalar1=rstd, scalar2=nmr,
                        op0=ALU.mult, op1=ALU.add)
            if apply_gamma:
                nc.vector.tensor_mul(out=y, in0=y, in1=gam_sb)
                nc.vector.tensor_add(out=y, in0=y, in1=bet_sb)
            nc.sync.dma_start(out=out[b, t * 128:(t + 1) * 128, :], in_=y)

        # ---------------- emission schedule ----------------
        load_inputs(0)
        load_weights(0)
        alloc_xf(0)
        prev = None
        for i in range(NPAIR):
            if i + 1 < NPAIR:
                load_weights(i + 1)
            if i == 6:
                load_inputs(1)
            if 3 <= i <= 6:
                load_wr_chunk(i - 3)
            emit_Q(i)
            if prev is not None:
                emit_pd(prev)
            emit_K(i)
            if prev is not None:
                emit_pbc(prev)
                emit_px(prev)
            emit_V(i)
            emit_scores(i)
            if i == 8:
                emit_rev(0)
                emit_ff(0)
                alloc_xf(1)
            elif i == 9:
                emit_stats(0)
            elif 10 <= i <= 13:
                emit_ln_chunk(0, i - 10)
            prev = i
        emit_pd(prev)
        emit_pbc(prev)
        emit_px(prev)
        emit_rev(1)
        emit_ff(1)
        emit_stats(1)
        for t in range(NT):
            emit_ln_chunk(1, t)

    nc.compile()
    _BUILD_CACHE[key] = nc
    return nc


def _prep_in_maps(inputs):
    import ml_dtypes

    def bf(x):
        return np.ascontiguousarray(np.asarray(x, dtype=np.float32).astype(
            ml_dtypes.bfloat16))

    def f32c(x):
        return np.ascontiguousarray(np.asarray(x, dtype=np.float32))

    text = bf(inputs["text"])
    image = bf(inputs["image"])
    wr = np.asarray(inputs["wr"], dtype=np.float64)
    bv = np.asarray(inputs["bv"], dtype=np.float64)
    br = np.asarray(inputs["br"], dtype=np.float64)
    breff = (br + bv @ wr).astype(np.float32)

    shared = {
        "wq": bf(inputs["wq"]), "wk": bf(inputs["wk"]), "wv": bf(inputs["wv"]),
        "wr": f32c(inputs["wr"]),
        "w1": bf(inputs["w1"]), "w2": bf(inputs["w2"]),
        "bq": f32c(inputs["bq"]), "bk": f32c(inputs["bk"]),
        "b1": f32c(inputs["b1"]), "b2": f32c(inputs["b2"]),
        "breff": breff, "gamma": f32c(inputs["gamma"]),
        "beta": f32c(inputs["beta"]),
    }
    in_maps = []
    for c in range(N_CORES):
        m = dict(shared)
        m["text"] = text[c * B:(c + 1) * B]
        m["image"] = image[c * B:(c + 1) * B]
        in_maps.append(m)
    return in_maps


def _needs_gamma(inputs):
    g = np.asarray(inputs["gamma"], dtype=np.float32)
    b = np.asarray(inputs["beta"], dtype=np.float32)
    return not (np.all(g == 1.0) and np.all(b == 0.0))


def kernel(**inputs) -> np.ndarray:
    _ensure_import_path()
    from concourse.bass_utils import run_bass_kernel_spmd

    nc = build_module(apply_gamma=_needs_gamma(inputs))
    in_maps = _prep_in_maps(inputs)
    res = run_bass_kernel_spmd(nc, in_maps, core_ids=list(range(N_CORES)))
    return np.concatenate([res.results[c]["out"] for c in range(N_CORES)], axis=0)


# revision 45
# speedup vs baseline: 1.0693x; 1.0693x over previous
"""Trainium2 Bass kernel for nn_AttentionLayer (cross-attention + FF + LayerNorm).

V2 strategy (data-parallel over batch, 2 per core):
  - bf16 on-chip activations/weights for projections + attention (full PE
    rate at any moving dim, 2x DVE, half DMA); f32r for the reversion
    (wr, xf) and the residual/LN path so the dominant error terms stay f32.
  - Inputs loaded FEATURE-major directly via DMA-transpose (XBAR), removing
    all phase-1 PE transposes and PSUM->SBUF copies.
  - wr loaded once into a const pool (not per batch), prefetched during
    attention of batch 0.
  - Phase 2 software-pipelined: the softmax tail of head i-1 (denominator,
    reciprocal, broadcast, V^T@E) is emitted inside head i's projection
    matmuls, so PE never stalls on Act/DVE round trips (keeps the PE
    p-state at full clock).
  - Softmax un-normalized in [key, query] layout; denominator via ones-row
    matmul; normalization folded into the PSUM->SBUF move of x (TT mult
    with the PE-broadcast reciprocal, both operands in PSUM).
  - LayerNorm: stats computed feature-major with ones-column matmuls
    (sum, sum-of-squares), rstd = exp(-0.5*ln(var+eps)) so every Act
    function lives in one act-table set (no table reloads); the normalize
    is fused into the PSUM->SBUF copies after the transpose back to
    token-major (per-partition scale/bias).
  - V bias folded into the reversion bias host-side: breff = br + bv @ wr.
"""

import os
import sys

import numpy as np

# ---- problem constants (hardcoded per contract) ----
B_TOTAL = 16
N_CORES = 8
B = B_TOTAL // N_CORES  # per-core batch
LT, DT = 512, 768       # text tokens / dim
LI, DI = 576, 1024      # image tokens / dim
H, NH, HD = 2048, 8, 256
FF = 128
ISCALE = 1.0 / 16.0     # 1/sqrt(HD)
NPAIR = B * NH          # 16 (batch, head) pairs per core
ITC = [(t, 128 if t < 4 else LI - 512) for t in range(5)]  # image tok chunks
NCD = DT // 128         # 6
NCI = DI // 128         # 8
NT = LT // 128          # 4

_BUILD_CACHE: dict = {}


def _ensure_import_path():
    try:
        import concourse  # noqa: F401
    except ModuleNotFoundError:
        for p in ("/opt/trn_rl_repo", "/root/.axon_site/_ro/trn_rl_repo"):
            if os.path.isdir(p) and p not in sys.path:
                sys.path.insert(0, p)


def build_module(apply_gamma: bool = False):
    key = ("v2", apply_gamma)
    if key in _BUILD_CACHE:
        return _BUILD_CACHE[key]
    _ensure_import_path()
    from contextlib import ExitStack

    import concourse.bacc as bacc
    import concourse.bass as bass  # noqa: F401
    import concourse.mybir as mybir
    import concourse.tile as tile
    from concourse.masks import make_identity

    f32 = mybir.dt.float32
    f32r = mybir.dt.float32r
    bf16 = mybir.dt.bfloat16
    AF = mybir.ActivationFunctionType
    ALU = mybir.AluOpType

    def r(ap):
        return ap.bitcast(f32r)

    nc = bacc.Bacc("TRN2", target_bir_lowering=False, debug=False, num_devices=N_CORES)

    text = nc.dram_tensor("text", [B, LT, DT], bf16, kind="ExternalInput").ap()
    image = nc.dram_tensor("image", [B, LI, DI], bf16, kind="ExternalInput").ap()
    wqp = nc.dram_tensor("wqp", [128, NH, NCD, HD], bf16,
                         kind="ExternalInput").ap()
    wkp = nc.dram_tensor("wkp", [128, NH, NCI, HD], bf16,
                         kind="ExternalInput").ap()
    wvp = nc.dram_tensor("wvp", [128, NH, NCI, HD], bf16,
                         kind="ExternalInput").ap()
    wrp = nc.dram_tensor("wrp", [128, H // 128, DT], f32,
                         kind="ExternalInput").ap()
    w1p = nc.dram_tensor("w1p", [128, NCD, FF], bf16, kind="ExternalInput").ap()
    w2 = nc.dram_tensor("w2", [FF, DT], bf16, kind="ExternalInput").ap()
    bqp = nc.dram_tensor("bqp", [128, H // 128], f32, kind="ExternalInput").ap()
    bkp = nc.dram_tensor("bkp", [128, H // 128], f32, kind="ExternalInput").ap()
    b1p = nc.dram_tensor("b1p", [128, 1], f32, kind="ExternalInput").ap()
    b2p = nc.dram_tensor("b2p", [128, NCD], f32, kind="ExternalInput").ap()
    breffp = nc.dram_tensor("breffp", [128, NCD], f32,
                            kind="ExternalInput").ap()
    gamp = nc.dram_tensor("gamp", [128, DT], f32, kind="ExternalInput").ap()
    betp = nc.dram_tensor("betp", [128, DT], f32, kind="ExternalInput").ap()
    out = nc.dram_tensor("out", [B, LT, DT], f32, kind="ExternalOutput").ap()

    def bcast_row(src, parts, n):
        return bass.AP(tensor=src.tensor, offset=src.offset, ap=[[0, parts], *src.ap])

    with tile.TileContext(nc) as tc, ExitStack() as ctx:
        const = ctx.enter_context(tc.tile_pool(name="const", bufs=1))
        ident = const.tile([128, 128], f32)
        make_identity(nc, ident)
        ident_r = const.tile([128, 128], f32)
        nc.vector.tensor_copy(out=r(ident_r), in_=ident)
        ones_col_b = const.tile([128, 1], bf16)
        nc.vector.memset(ones_col_b, 1.0)
        ones_row_b = const.tile([1, 128], bf16)
        nc.vector.memset(ones_row_b, 1.0)
        ones_tmp = const.tile([128, 16], f32)
        nc.vector.memset(ones_tmp, 1.0)
        ones16_f = const.tile([128, 16], f32)
        nc.vector.tensor_copy(out=r(ones16_f), in_=ones_tmp)
        ones16_b = const.tile([128, 16], bf16)
        nc.vector.memset(ones16_b, 1.0)
        # float with bit pattern 0x5f3759df (quake rsqrt magic)
        magic4 = const.tile([128, NT], f32)
        nc.vector.memset(magic4, float(np.uint32(0x5F3759DF).view(np.float32)))
        # dummy activation: forces the act-table load off the critical path
        warm = const.tile([1, 1], f32)
        nc.scalar.activation(out=warm, in_=magic4[0:1, 0:1], func=AF.Exp,
                             scale=0.0)

        bq_sb = const.tile([128, H // 128], f32)
        bk_sb = const.tile([128, H // 128], f32)
        b1_sb = const.tile([128, 1], f32)
        b2_sb = const.tile([128, NCD], f32)
        breff_sb = const.tile([128, NCD], f32)
        w1_sb = const.tile([128, NCD, FF], bf16)
        w2_sb = const.tile([128, DT], bf16)
        wr_sb = const.tile([128, H // 128, DT], f32)  # loaded in 4 chunks mid-flight
        gam_sb = bet_sb = None
        if apply_gamma:
            gam_sb = const.tile([128, DT], f32)
            bet_sb = const.tile([128, DT], f32)

        def load_consts_late():
            nc.sync.dma_start(out=b1_sb, in_=b1p)
            nc.sync.dma_start(out=b2_sb, in_=b2p)
            nc.sync.dma_start(out=breff_sb, in_=breffp)
            nc.sync.dma_start(out=w1_sb, in_=w1p)
            nc.sync.dma_start(out=w2_sb, in_=w2)
            if apply_gamma:
                nc.sync.dma_start(out=gam_sb, in_=gamp)
                nc.sync.dma_start(out=bet_sb, in_=betp)

        psum = ctx.enter_context(tc.tile_pool(name="psum", bufs=6, space="PSUM"))
        psd = ctx.enter_context(tc.tile_pool(name="psd", bufs=2, space="PSUM"))
        tfp = ctx.enter_context(tc.tile_pool(name="tfp", bufs=2))
        hwp = ctx.enter_context(tc.tile_pool(name="hwp", bufs=2))
        atp = ctx.enter_context(tc.tile_pool(name="atp", bufs=2))
        xfp = ctx.enter_context(tc.tile_pool(name="xfp", bufs=1))
        ofp = ctx.enter_context(tc.tile_pool(name="ofp", bufs=1))
        p5p = ctx.enter_context(tc.tile_pool(name="p5p", bufs=2))

        TFI: dict = {}
        W: dict = {}
        S: dict = {}
        XF: dict = {}
        OF: dict = {}
        SQ: dict = {}

        def load_weights(i, eng=None):
            eng = eng or nc.sync
            h = i % NH
            wq_h = hwp.tile([128, NCD, HD], bf16, tag="wqh", name="wqh")
            eng.dma_start(out=wq_h, in_=wqp[:, h])
            wk_h = hwp.tile([128, NCI, HD], bf16, tag="wkh", name="wkh")
            eng.dma_start(out=wk_h, in_=wkp[:, h])
            wv_h = hwp.tile([128, NCI, HD], bf16, tag="wvh", name="wvh")
            eng.dma_start(out=wv_h, in_=wvp[:, h])
            W[i] = (wq_h, wk_h, wv_h)

        def load_wr_chunk(cc):
            nc.sync.dma_start(
                out=r(wr_sb[:, cc * 4:(cc + 1) * 4, :]),
                in_=r(wrp[:, cc * 4:(cc + 1) * 4, :]))

        def alloc_xf(b):
            XF[b] = [xfp.tile([128, LT], f32, tag=f"xf{c}", name=f"xf{c}")
                     for c in range(H // 128)]

        def emit_Q(i):
            b, h = divmod(i, NH)
            tfs, _ = TFI[b]
            wq_h, _, _ = W[i]
            q_h = atp.tile([128, 2, LT], bf16, tag="qh", name="qh")
            for m in range(2):
                pq = psum.tile([128, LT], f32, tag="ps512", name="ps512")
                for c in range(NCD):
                    nc.tensor.matmul(pq, wq_h[:, c, m * 128:(m + 1) * 128], tfs[c],
                                     start=(c == 0), stop=(c == NCD - 1))
                nc.scalar.activation(
                    out=q_h[:, m, :], in_=pq, func=AF.Identity,
                    bias=bq_sb[:, h * 2 + m:h * 2 + m + 1], scale=1.0)
            S.setdefault(i, {})["q"] = q_h

        def emit_K(i, c_outer=False):
            b, h = divmod(i, NH)
            _, ifs = TFI[b]
            _, wk_h, _ = W[i]
            k_h = atp.tile([128, 2, LI], bf16, tag="kh", name="kh")
            if c_outer:
                pks = {}
                for m in range(2):
                    for n in range(2):
                        pks[(m, n)] = psum.tile([128, 288], f32, tag="ps512",
                                                name="ps512")
                for c in range(NCI):
                    for m in range(2):
                        for n in range(2):
                            nc.tensor.matmul(
                                pks[(m, n)], wk_h[:, c, m * 128:(m + 1) * 128],
                                ifs[c][:, n * 288:(n + 1) * 288],
                                start=(c == 0), stop=(c == NCI - 1))
                for m in range(2):
                    for n in range(2):
                        nc.scalar.activation(
                            out=k_h[:, m, n * 288:(n + 1) * 288],
                            in_=pks[(m, n)], func=AF.Identity,
                            bias=bk_sb[:, h * 2 + m:h * 2 + m + 1], scale=1.0)
            else:
                for m in range(2):
                    for n in range(2):
                        pk = psum.tile([128, 288], f32, tag="ps512", name="ps512")
                        for c in range(NCI):
                            nc.tensor.matmul(
                                pk, wk_h[:, c, m * 128:(m + 1) * 128],
                                ifs[c][:, n * 288:(n + 1) * 288],
                                start=(c == 0), stop=(c == NCI - 1))
                        nc.scalar.activation(
                            out=k_h[:, m, n * 288:(n + 1) * 288], in_=pk,
                            func=AF.Identity,
                            bias=bk_sb[:, h * 2 + m:h * 2 + m + 1], scale=1.0)
            S[i]["k"] = k_h

        def emit_V(i, c_outer=False):
            b, h = divmod(i, NH)
            _, ifs = TFI[b]
            _, _, wv_h = W[i]
            v_h = atp.tile([128, 5, HD], bf16, tag="vh", name="vh")
            if c_outer:
                pvs = {}
                for t, pt in ITC:
                    pvs[t] = psum.tile([128, HD], f32, tag="ps512", name="ps512")
                for c in range(NCI):
                    for t, pt in ITC:
                        nc.tensor.matmul(
                            pvs[t][:pt], ifs[c][:, t * 128:t * 128 + pt],
                            wv_h[:, c, :],
                            start=(c == 0), stop=(c == NCI - 1))
                for t, pt in ITC:
                    nc.vector.tensor_copy(out=v_h[:pt, t, :], in_=pvs[t][:pt])
            else:
                for t, pt in ITC:
                    pv = psum.tile([128, HD], f32, tag="ps512", name="ps512")
                    for c in range(NCI):
                        nc.tensor.matmul(
                            pv[:pt], ifs[c][:, t * 128:t * 128 + pt],
                            wv_h[:, c, :],
                            start=(c == 0), stop=(c == NCI - 1))
                    nc.vector.tensor_copy(out=v_h[:pt, t, :], in_=pv[:pt])
            S[i]["v"] = v_h

        def emit_scores(i):
            q_h, k_h = S[i]["q"], S[i]["k"]
            e_f = atp.tile([128, 5, LT], bf16, tag="ef", name="ef")
            for t, pt in ITC:
                ps_s = psum.tile([128, LT], f32, tag="ps512", name="ps512")
                for m in range(2):
                    nc.tensor.matmul(
                        ps_s[:pt], k_h[:, m, t * 128:t * 128 + pt], q_h[:, m, :],
                        start=(m == 0), stop=(m == 1))
                nc.scalar.activation(
                    out=e_f[:pt, t, :], in_=ps_s[:pt], func=AF.Exp, scale=ISCALE)
            S[i]["e"] = e_f

        def emit_pd(i):
            e_f = S[i]["e"]
            pd2 = psd.tile([1, LT], f32, tag="psd", name="psd")
            for t, pt in ITC:
                nc.tensor.matmul(pd2[0:1, :], ones_col_b[:pt], e_f[:pt, t, :],
                                 start=(t == 0), stop=(t == 4))
            recip = atp.tile([1, LT], bf16, tag="recip", name="recip")
            with nc.allow_low_precision(reason="softmax recip feeds bf16 matmul"):
                nc.vector.reciprocal(out=recip, in_=pd2[0:1, :])
            S[i]["recip"] = recip

        def emit_pbc(i):
            pbc = psum.tile([128, LT], f32, tag="ps512", name="ps512")
            nc.tensor.matmul(pbc, ones_row_b, S[i]["recip"])
            bcast = atp.tile([128, LT], bf16, tag="bcast", name="bcast")
            nc.scalar.activation(out=bcast, in_=pbc, func=AF.Copy)
            S[i]["bcast"] = bcast

        def emit_px(i):
            b, h = divmod(i, NH)
            v_h, e_f, bcast = S[i]["v"], S[i]["e"], S[i]["bcast"]
            for m in range(2):
                px = psum.tile([128, LT], f32, tag="ps512", name="ps512")
                for t, pt in ITC:
                    nc.tensor.matmul(
                        px, v_h[:pt, t, m * 128:(m + 1) * 128], e_f[:pt, t, :],
                        start=(t == 0), stop=(t == 4))
                nc.vector.tensor_mul(out=r(XF[b][h * 2 + m]), in0=px, in1=bcast)
            S[i] = None  # release references

        def emit_rev(b, split_first=None):
            ofs = [ofp.tile([128, LT], f32, tag=f"of{m}", name=f"of{m}")
                   for m in range(NCD)]
            ofb = [ofp.tile([128, LT], bf16, tag=f"ofb{m}", name=f"ofb{m}")
                   for m in range(NCD)]
            NC16 = H // 128
            for m in range(NCD):
                po = psum.tile([128, LT], f32, tag="ps512", name="ps512")
                if m == 0 and split_first is not None:
                    for c in range(NC16 - 2):
                        nc.tensor.matmul(po,
                                         r(wr_sb[:, c, m * 128:(m + 1) * 128]),
                                         r(XF[b][c]), start=(c == 0), stop=False)
                    split_first()
                    for c in (NC16 - 2, NC16 - 1):
                        nc.tensor.matmul(po,
                                         r(wr_sb[:, c, m * 128:(m + 1) * 128]),
                                         r(XF[b][c]), start=False,
                                         stop=(c == NC16 - 1))
                else:
                    for c in range(NC16):
                        nc.tensor.matmul(po,
                                         r(wr_sb[:, c, m * 128:(m + 1) * 128]),
                                         r(XF[b][c]),
                                         start=(c == 0), stop=(c == NC16 - 1))
                nc.scalar.activation(
                    out=r(ofs[m]), in_=po, func=AF.Identity,
                    bias=breff_sb[:, m:m + 1], scale=1.0)
                nc.vector.tensor_scalar(
                    out=ofb[m], in0=po, scalar1=breff_sb[:, m:m + 1], scalar2=None,
                    op0=ALU.add)
            OF[b] = (ofs, ofb)

        def emit_ff_stats(b):
            ofs, ofb = OF[b]
            i32 = mybir.dt.int32
            ph = psum.tile([128, LT], f32, tag="ps512", name="ps512")
            for c in range(NCD):
                nc.tensor.matmul(ph, w1_sb[:, c, :], ofb[c],
                                 start=(c == 0), stop=(c == NCD - 1))
            h_sb = p5p.tile([128, LT], bf16, tag="hsb", name="hsb")
            nc.scalar.activation(out=h_sb[:, 0:256], in_=ph[:, 0:256],
                                 func=AF.Relu, bias=b1_sb, scale=1.0)
            nc.vector.tensor_scalar(out=h_sb[:, 256:512], in0=ph[:, 256:512],
                                    scalar1=b1_sb, scalar2=0.0, op0=ALU.add,
                                    op1=ALU.max)
            pstA = psd.tile([16, LT], f32, tag="psd", name="psd")
            pstB = psd.tile([16, LT], f32, tag="psd", name="psd")

            def pf_mm(m):
                pf = psum.tile([128, LT], f32, tag="ps512", name="ps512")
                nc.tensor.matmul(pf, w2_sb[:, m * 128:(m + 1) * 128], h_sb)
                # fused: ofs[m] = (pf + b2[m]) + ofs[m]   (residual + bias)
                nc.vector.scalar_tensor_tensor(
                    out=r(ofs[m]), in0=pf, scalar=b2_sb[:, m:m + 1],
                    in1=ofs[m], op0=ALU.add, op1=ALU.add)
                if m % 2 == 0:
                    nc.vector.tensor_mul(out=ofb[m], in0=ofs[m], in1=ofs[m])
                else:
                    nc.scalar.activation(out=ofb[m], in_=ofs[m],
                                         func=AF.Square, scale=1.0)
                SQ.setdefault(b, []).append(ofb[m])

            def stA(m):
                nc.tensor.matmul(pstA, r(ones16_f), r(ofs[m]),
                                 start=(m == 0), stop=(m == NCD - 1))

            def stB(m):
                nc.tensor.matmul(pstB, ones16_b, ofb[m],
                                 start=(m == 0), stop=(m == NCD - 1))

            # interleave stats accumulation behind the pf matmuls so PE never
            # waits on the stt/sq chains
            pf_mm(0)
            pf_mm(1)
            stA(0)
            pf_mm(2)
            stA(1)
            stB(0)
            pf_mm(3)
            stA(2)
            stB(1)
            pf_mm(4)
            stA(3)
            stB(2)
            pf_mm(5)
            stA(4)
            stB(3)
            stA(5)
            stB(4)
            stB(5)

            srowA = ofp.tile([16, LT], f32, tag="srowA", name="srowA")
            srowB = ofp.tile([16, LT], f32, tag="srowB", name="srowB")
            nc.vector.tensor_copy(out=r(srowA), in_=pstA)
            nc.scalar.activation(out=r(srowB), in_=pstB, func=AF.Copy)
            mq4 = p5p.tile([128, NT, 32], f32, tag="mq4", name="mq4")
            ptc = psum.tile([128, NT, 32], f32, tag="ps512", name="ps512")
            for t in range(NT):
                nc.tensor.transpose(r(ptc[:, t, 0:16]),
                                    r(srowA[:, t * 128:(t + 1) * 128]),
                                    r(ident_r[:16, :16]))
                nc.tensor.transpose(r(ptc[:, t, 16:32]),
                                    r(srowB[:, t * 128:(t + 1) * 128]),
                                    r(ident_r[:16, :16]))
            nc.scalar.activation(out=mq4, in_=ptc, func=AF.Identity,
                                 scale=1.0 / DT)
            mu4 = mq4[:, :, 0]
            q4 = mq4[:, :, 16]
            musq = p5p.tile([128, NT], f32, tag="musq4", name="musq4")
            nc.vector.tensor_mul(out=musq, in0=mu4, in1=mu4)
            v4 = p5p.tile([128, NT], f32, tag="v4", name="v4")
            nc.vector.tensor_sub(out=v4, in0=q4, in1=musq)
            # rstd = rsqrt(v4): quake bit-trick seed + Newton steps, DVE only
            y4 = p5p.tile([128, NT], f32, tag="y4", name="y4")
            sh4 = p5p.tile([128, NT], f32, tag="sh4", name="sh4")
            nc.vector.tensor_scalar(
                out=sh4.bitcast(i32), in0=v4.bitcast(i32), scalar1=1,
                scalar2=None, op0=ALU.logical_shift_right)
            nc.vector.tensor_sub(out=y4.bitcast(i32), in0=magic4.bitcast(i32),
                                 in1=sh4.bitcast(i32))
            t14 = p5p.tile([128, NT], f32, tag="t14", name="t14")
            for _ in range(1):
                nc.vector.tensor_mul(out=t14, in0=y4, in1=y4)
                nc.vector.tensor_mul(out=t14, in0=t14, in1=v4)
                nc.vector.tensor_scalar(out=t14, in0=t14, scalar1=-0.5,
                                        scalar2=1.5, op0=ALU.mult, op1=ALU.add)
                nc.vector.tensor_mul(out=y4, in0=y4, in1=t14)
            nmr4 = p5p.tile([128, NT], f32, tag="nmr4", name="nmr4")
            nc.vector.tensor_mul(out=nmr4, in0=mu4, in1=y4)
            nc.vector.tensor_scalar(out=nmr4, in0=nmr4, scalar1=-1.0,
                                    scalar2=None, op0=ALU.mult)
            stats = [(y4[:, t:t + 1], nmr4[:, t:t + 1]) for t in range(NT)]
            OF[b] = (ofs, stats)

        def emit_ln_chunk(b, t):
            ofs, stats = OF[b]
            rstd, nmr = stats[t]
            y = p5p.tile([128, DT], f32, tag=f"y{t % 2}", name=f"y{t % 2}")
            for half in range(2):
                ptr_ = psum.tile([128, 384], f32, tag="ps512", name="ps512")
                for j in range(3):
                    c = half * 3 + j
                    nc.tensor.transpose(
                        r(ptr_[:, j * 128:(j + 1) * 128]),
                        r(ofs[c][:, t * 128:(t + 1) * 128]), r(ident_r))
                dst = y[:, half * 384:(half + 1) * 384]
                if half == 0:
                    nc.scalar.activation(out=dst, in_=ptr_, func=AF.Identity,
                                         bias=nmr, scale=rstd)
                else:
                    nc.vector.tensor_scalar(
                        out=dst, in0=ptr_, scalar1=rstd, scalar2=nmr,
                        op0=ALU.mult, op1=ALU.add)
                if apply_gamma:
                    nc.vector.tensor_mul(out=dst, in0=dst,
                                         in1=gam_sb[:, half * 384:(half + 1) * 384])
                    nc.vector.tensor_add(out=dst, in0=dst,
                                         in1=bet_sb[:, half * 384:(half + 1) * 384])
                if b == 1 and t == NT - 1:
                    eng = nc.scalar if half == 0 else nc.sync
                    eng.dma_start(
                        out=out[b, t * 128:(t + 1) * 128,
                                half * 384:(half + 1) * 384],
                        in_=dst)
            if not (b == 1 and t == NT - 1):
                nc.sync.dma_start(out=out[b, t * 128:(t + 1) * 128, :], in_=y)

        def load_text(b):
            tf_a = tfp.tile([128, 3, LT], bf16, tag="textfa", name="textfa")
            nc.sync.dma_start_transpose(out=tf_a, in_=text[b, :, 0:384])
            tf_b = tfp.tile([128, 3, LT], bf16, tag="textfb", name="textfb")
            nc.sync.dma_start_transpose(out=tf_b, in_=text[b, :, 384:768])
            return [tf_a[:, c, :] for c in range(3)] + \
                   [tf_b[:, c, :] for c in range(3)]

        def alloc_image():
            return (tfp.tile([128, 4, LI], bf16, tag="imgfa", name="imgfa"),
                    tfp.tile([128, 4, LI], bf16, tag="imgfb", name="imgfb"))

        def load_image_half(if_ab, b, half):
            if half == 0:
                nc.sync.dma_start_transpose(out=if_ab[0], in_=image[b, :, 0:512])
            else:
                nc.sync.dma_start_transpose(out=if_ab[1],
                                            in_=image[b, :, 512:1024])

        def load_image(b):
            if_ab = alloc_image()
            load_image_half(if_ab, b, 0)
            load_image_half(if_ab, b, 1)
            return [if_ab[0][:, c, :] for c in range(4)] + \
                   [if_ab[1][:, c, :] for c in range(4)]

        # ---------------- emission schedule ----------------
        # startup: image + K/V weights first (the longer pole; block 0 runs
        # K,V before Q); single big DMA-transposes per input tensor.
        wk_h0 = hwp.tile([128, NCI, HD], bf16, tag="wkh", name="wkh")
        nc.sync.dma_start(out=wk_h0[:, 0:4, :], in_=wkp[:, 0, 0:4, :])
        if0 = alloc_image()
        load_image_half(if0, 0, 0)
        nc.sync.dma_start(out=wk_h0[:, 4:8, :], in_=wkp[:, 0, 4:8, :])
        load_image_half(if0, 0, 1)
        wv_h0 = hwp.tile([128, NCI, HD], bf16, tag="wvh", name="wvh")
        nc.sync.dma_start(out=wv_h0, in_=wvp[:, 0])
        ifs0 = [if0[0][:, c, :] for c in range(4)] + \
               [if0[1][:, c, :] for c in range(4)]
        nc.scalar.dma_start(out=bk_sb, in_=bkp)
        wq_h0 = hwp.tile([128, NCD, HD], bf16, tag="wqh", name="wqh")
        nc.sync.dma_start(out=wq_h0, in_=wqp[:, 0])
        tfs0 = load_text(0)
        nc.scalar.dma_start(out=bq_sb, in_=bqp)
        TFI[0] = (tfs0, ifs0)
        W[0] = (wq_h0, wk_h0, wv_h0)
        alloc_xf(0)
        prev = None
        for i in range(NPAIR - 1):
            if i > 0:
                load_weights(i + 1)
            if i == 2:
                load_consts_late()
            if i == 6:
                TFI[1] = (load_text(1), load_image(1))
            if 3 <= i <= 6:
                load_wr_chunk(i - 3)
            if prev is None:
                S[i] = {}
                emit_K(i, c_outer=True)
                emit_V(i, c_outer=True)
                emit_Q(i)
            else:
                emit_Q(i)
                emit_pd(prev)
                emit_K(i)
                emit_pbc(prev)
                emit_px(prev)
                emit_V(i)
            emit_scores(i)
            if i == 0:
                load_weights(1)
            if i == 8:
                emit_rev(0)
                emit_ff_stats(0)
                alloc_xf(1)
            elif 9 <= i <= 12:
                emit_ln_chunk(0, i - 9)
            prev = i
        # ---- block 15: scores before V so exp(15) finishes during V;
        # reversion's first group splits around the pair-15 softmax tail ----
        i = NPAIR - 1
        emit_Q(i)
        emit_pd(prev)
        emit_K(i)
        emit_pbc(prev)
        emit_px(prev)
        emit_scores(i)
        emit_V(i)
        emit_pd(i)

        def _tail15():
            emit_pbc(i)
            emit_px(i)

        emit_rev(1, split_first=_tail15)
        emit_ff_stats(1)
        for t in range(NT):
            emit_ln_chunk(1, t)

    nc.compile()
    _BUILD_CACHE[key] = nc
    return nc


def _prep_in_maps(inputs):
    import ml_dtypes

    def bf(x):
        return np.ascontiguousarray(np.asarray(x, dtype=np.float32).astype(
            ml_dtypes.bfloat16))

    def f32c(x):
        return np.ascontiguousarray(np.asarray(x, dtype=np.float32))

    def headmajor(w, din):
        # [din, H] -> [128, NH, din//128, HD] (per-partition contiguous runs)
        return np.ascontiguousarray(
            np.asarray(w).reshape(din // 128, 128, NH, HD).transpose(1, 2, 0, 3))

    text = bf(inputs["text"])
    image = bf(inputs["image"])
    wr = np.asarray(inputs["wr"], dtype=np.float64)
    bv = np.asarray(inputs["bv"], dtype=np.float64)
    br = np.asarray(inputs["br"], dtype=np.float64)
    breff = (br + bv @ wr).astype(np.float32)
    w1 = bf(inputs["w1"])
    gamma = f32c(inputs["gamma"])
    beta = f32c(inputs["beta"])

    shared = {
        "wqp": headmajor(bf(inputs["wq"]), DT),
        "wkp": headmajor(bf(inputs["wk"]), DI),
        "wvp": headmajor(bf(inputs["wv"]), DI),
        "wrp": np.ascontiguousarray(
            f32c(inputs["wr"]).reshape(H // 128, 128, DT).transpose(1, 0, 2)),
        "w1p": np.ascontiguousarray(
            w1.reshape(NCD, 128, FF).transpose(1, 0, 2)),
        "w2": bf(inputs["w2"]),
        "bqp": np.ascontiguousarray(
            f32c(inputs["bq"]).reshape(H // 128, 128).T),
        "bkp": np.ascontiguousarray(
            f32c(inputs["bk"]).reshape(H // 128, 128).T),
        "b1p": np.ascontiguousarray(f32c(inputs["b1"]).reshape(128, 1)),
        "b2p": np.ascontiguousarray(f32c(inputs["b2"]).reshape(NCD, 128).T),
        "breffp": np.ascontiguousarray(breff.reshape(NCD, 128).T),
        "gamp": np.ascontiguousarray(np.broadcast_to(gamma, (128, DT))),
        "betp": np.ascontiguousarray(np.broadcast_to(beta, (128, DT))),
    }
    in_maps = []
    for c in range(N_CORES):
        m = dict(shared)
        m["text"] = text[c * B:(c + 1) * B]
        m["image"] = image[c * B:(c + 1) * B]
        in_maps.append(m)
    return in_maps


def _needs_gamma(inputs):
    g = np.asarray(inputs["gamma"], dtype=np.float32)
    b = np.asarray(inputs["beta"], dtype=np.float32)
    return not (np.all(g == 1.0) and np.all(b == 0.0))


def kernel(**inputs) -> np.ndarray:
    _ensure_import_path()
    from concourse.bass_utils import run_bass_kernel_spmd

    nc = build_module(apply_gamma=_needs_gamma(inputs))
    in_maps = _prep_in_maps(inputs)
    res = run_bass_kernel_spmd(nc, in_maps, core_ids=list(range(N_CORES)))
    return np.concatenate([res.results[c]["out"] for c in range(N_CORES)], axis=0)
